# revision 1
# baseline (speedup 1.0000x reference)
"""Trainium2 Bass kernel for BiLinearSigmoidAttention.

Reference math (per batch b, with L = length[b]):
    qn = l2norm(query), cn = l2norm(context)
    raw[q,k] = qn[q] . cn[k]            (masked: k >= L -> -1e30)
    sig = sigmoid(raw)
    den[q] = max(sum_k sig[q,k], 1)
    scores[q,k] = sig[q,k] / den[q]     (rows q >= L zeroed)
    att[q,:] = sum_k scores[q,k] * context[k,:]
    out = concat([qn, att], -1)
returns (out [B,S,2D], scores [B,S,S])

Device mapping (8 NeuronCores, pure data parallel over B=32 -> 4 per core):
  - mm1 computes scoresT [k_part, q_free] so the length mask is a
    per-partition bias fused into the ACT sigmoid.
  - denominator = ones-column matmuls sharing mm2's loaded weights,
    accumulated per q-block into tiny PSUM tiles (partition-major).
  - scores output produced by PE transposes of sigT, scaled by
    w = qmask/den during PSUM->SBUF eviction.
  - matmuls run as float32r (full-rate fp32); transposes as fp32.
"""

import numpy as np

import concourse.bacc as bacc
import concourse.mybir as mybir
import concourse.tile as tile
from concourse.bass_utils import run_bass_kernel_spmd

B, S, D = 32, 1024, 512
NCORES = 8
BPC = B // NCORES          # batches per core
P = 128                    # partitions
NT = S // P                # 8 s-tiles
ND = D // P                # 4 d-chunks
NEG = np.float32(-1e30)

F32 = mybir.dt.float32
F32R = mybir.dt.float32r
AF = mybir.ActivationFunctionType
ALU = mybir.AluOpType
AX = mybir.AxisListType


def _r(ap):
    """View an fp32 AP as float32r for full-rate PE matmuls."""
    return ap.bitcast(F32R)


def build_kernel():
    nc = bacc.Bacc("TRN2", target_bir_lowering=False, debug=False)

    q_d = nc.dram_tensor("query", [BPC, S, D], F32, kind="ExternalInput")
    c_d = nc.dram_tensor("context", [BPC, S, D], F32R, kind="ExternalInput")
    # keybias[b, p, kt] = 0 if kt*P+p < L else -1e30
    kb_d = nc.dram_tensor("keybias", [BPC, P, NT], F32, kind="ExternalInput")
    # qmask[b, p, qb] = 1 if qb*P+p < L else 0
    qm_d = nc.dram_tensor("qmask", [BPC, P, NT], F32, kind="ExternalInput")
    id_d = nc.dram_tensor("identity", [P, P], F32, kind="ExternalInput")
    idr_d = nc.dram_tensor("identity_r", [P, P], F32R, kind="ExternalInput")
    on_d = nc.dram_tensor("ones", [P, 2], F32R, kind="ExternalInput")
    out_d = nc.dram_tensor("out", [BPC, S, 2 * D], F32, kind="ExternalOutput")
    sc_d = nc.dram_tensor("scores", [BPC, S, S], F32, kind="ExternalOutput")

    with tile.TileContext(nc) as tc:
        _body(tc, q_d, c_d, kb_d, qm_d, id_d, idr_d, on_d, out_d, sc_d)
    nc.compile()
    return nc


def _body(tc, q_d, c_d, kb_d, qm_d, id_d, idr_d, on_d, out_d, sc_d):
    import os

    PHASE = int(os.environ.get("KERNEL_PHASE", "4"))
    nc = tc.nc
    from contextlib import ExitStack

    ctx = ExitStack()
    with ctx:
        const = ctx.enter_context(tc.tile_pool(name="const", bufs=1))
        qpool = ctx.enter_context(tc.tile_pool(name="q", bufs=2))
        cpool = ctx.enter_context(tc.tile_pool(name="c", bufs=2))
        tpool = ctx.enter_context(tc.tile_pool(name="t", bufs=1))
        sgpool = ctx.enter_context(tc.tile_pool(name="sg", bufs=1))
        mpool = ctx.enter_context(tc.tile_pool(name="m", bufs=2))
        spool = ctx.enter_context(tc.tile_pool(name="s", bufs=3))
        opool = ctx.enter_context(tc.tile_pool(name="o", bufs=3))
        ps1 = ctx.enter_context(tc.tile_pool(name="ps1", bufs=2, space="PSUM"))
        pst = ctx.enter_context(tc.tile_pool(name="pst", bufs=2, space="PSUM"))
        ps2 = ctx.enter_context(tc.tile_pool(name="ps2", bufs=2, space="PSUM"))
        psd = ctx.enter_context(tc.tile_pool(name="psd", bufs=2, space="PSUM"))

        ident = const.tile([P, P], F32, tag="ident")
        identr = const.tile([P, P], F32R, tag="identr")
        ones = const.tile([P, 2], F32R, tag="ones")
        nc.sync.dma_start(ident[:], id_d[:])
        nc.sync.dma_start(identr[:], idr_d[:])
        nc.sync.dma_start(ones[:], on_d[:])

        for b in range(BPC):
            # ---- load ----
            qt = qpool.tile([P, NT, D], F32, tag="qt")       # qn (in-place)
            ct = cpool.tile([P, NT, D], F32R, tag="ct")       # raw context
            kb = mpool.tile([P, NT], F32, tag="kb")
            qm = mpool.tile([P, NT], F32, tag="qm")
            nc.sync.dma_start(qt[:], q_d[b].rearrange("(t p) d -> p t d", p=P))
            nc.sync.dma_start(ct[:], c_d[b].rearrange("(t p) d -> p t d", p=P))
            nc.sync.dma_start(kb[:], kb_d[b])
            nc.sync.dma_start(qm[:], qm_d[b])

            # ---- norms ----
            ssq = mpool.tile([P, 2 * NT], F32, tag="ssq")
            inv = mpool.tile([P, 2 * NT], F32, tag="inv")
            for t in range(NT):
                scr = spool.tile([P, D], F32, tag="scr")
                nc.vector.tensor_mul(scr[:], qt[:, t], qt[:, t])
                nc.vector.reduce_sum(ssq[:, t : t + 1], scr[:], axis=AX.X)
                scr2 = spool.tile([P, D], F32, tag="scr2")
                nc.scalar.activation(
                    scr2[:], ct[:, t], AF.Square,
                    accum_out=ssq[:, NT + t : NT + t + 1],
                )
            # inv = 1/sqrt(ssq)  (norms are >0 with randn inputs)
            nrm = mpool.tile([P, 2 * NT], F32, tag="nrm")
            nc.scalar.activation(nrm[:], ssq[:], AF.Sqrt)
            nc.vector.reciprocal(inv[:], nrm[:])

            # ---- qn in place, store first half of out ----
            for t in range(NT):
                nc.vector.tensor_scalar_mul(qt[:, t], qt[:, t], inv[:, t : t + 1])
            nc.sync.dma_start(
                out_d[b, :, 0:D].rearrange("(t p) d -> p t d", p=P), qt[:]
            )

            if PHASE < 2:
                continue
            # ---- transposes: qT[d, s] and cnT[d, s] ----
            qT = tpool.tile([P, ND, S], F32R, tag="qT")
            cT = tpool.tile([P, ND, S], F32R, tag="cT")
            for t in range(NT):
                pq = pst.tile([P, ND, P], F32, tag="pt")
                pc = pst.tile([P, ND, P], F32R, tag="pt")
                for dch in range(ND):
                    nc.tensor.transpose(
                        pq[:, dch], qt[:, t, dch * P : (dch + 1) * P], ident[:]
                    )
                    nc.tensor.transpose(
                        pc[:, dch], ct[:, t, dch * P : (dch + 1) * P], identr[:]
                    )
                nc.scalar.copy(qT[:, :, t * P : (t + 1) * P], pq[:])
                nc.vector.tensor_copy(cT[:, :, t * P : (t + 1) * P], pc[:])

            if PHASE < 3:
                continue
            # ---- mm1: sigT[k, q] = sigmoid(cnT.T @ qT + keybias) ----
            sg = sgpool.tile([P, NT, S], F32R, tag="sg")
            for kt in range(NT):
                for qc in range(2):
                    acc = ps1.tile([P, 512], F32, tag="acc")
                    for dch in range(ND):
                        nc.tensor.matmul(
                            acc[:],
                            cT[:, dch, kt * P : (kt + 1) * P],
                            qT[:, dch, qc * 512 : (qc + 1) * 512],
                            start=(dch == 0),
                            stop=(dch == ND - 1),
                        )
                    # context l2-normalization folds in as the per-k scale
                    nc.scalar.activation(
                        sg[:, kt, qc * 512 : (qc + 1) * 512], acc[:],
                        AF.Sigmoid, bias=kb[:, kt : kt + 1],
                        scale=inv[:, NT + kt : NT + kt + 1],
                    )

            if PHASE < 4:
                continue
            # ---- per q-block: denominator, attended, scores out ----
            for qb in range(NT):
                att = ps2.tile([P, 512], F32, tag="att")
                dn = psd.tile([P, 2], F32, tag="dn")
                for kt in range(NT):
                    sgblk = sg[:, kt, qb * P : (qb + 1) * P]
                    nc.tensor.matmul(
                        att[:], sgblk, ct[:, kt],
                        start=(kt == 0), stop=(kt == NT - 1),
                    )
                    nc.tensor.matmul(
                        dn[:], sgblk, ones[:],
                        start=(kt == 0), stop=(kt == NT - 1),
                    )
                # w = qmask / max(den, 1)
                w = mpool.tile([P, 1], F32, tag="w")
                nc.vector.tensor_scalar_max(w[:], dn[:, 0:1], 1.0)
                nc.vector.reciprocal(w[:], w[:])
                nc.vector.tensor_mul(w[:], w[:], qm[:, qb : qb + 1])

                ao = opool.tile([P, D], F32, tag="ao")
                nc.vector.tensor_scalar_mul(ao[:], att[:], w[:])
                nc.sync.dma_start(out_d[b, qb * P : (qb + 1) * P, D : 2 * D], ao[:])

                so = opool.tile([P, S], F32, tag="so")
                for kg in range(2):
                    pt = pst.tile([P, 4, P], F32R, tag="pt")
                    for j in range(4):
                        kt = kg * 4 + j
                        nc.tensor.transpose(
                            pt[:, j], sg[:, kt, qb * P : (qb + 1) * P], identr[:]
                        )
                    eng = nc.scalar if kg == 0 else nc.vector
                    if kg == 0:
                        nc.scalar.activation(
                            so[:, kg * 512 : (kg + 1) * 512], pt[:],
                            AF.Copy, scale=w[:],
                        )
                    else:
                        nc.vector.tensor_scalar_mul(
                            so[:, kg * 512 : (kg + 1) * 512], pt[:], w[:]
                        )
                nc.sync.dma_start(sc_d[b, qb * P : (qb + 1) * P, :], so[:])


_NC_CACHE = {}


def _get_nc():
    if "nc" not in _NC_CACHE:
        _NC_CACHE["nc"] = build_kernel()
    return _NC_CACHE["nc"]


def kernel(context, query, length):
    context = np.ascontiguousarray(np.asarray(context, dtype=np.float32))
    query = np.ascontiguousarray(np.asarray(query, dtype=np.float32))
    length = np.asarray(length).astype(np.int64)

    iot = np.arange(S)
    keymask = iot[None, :] < length[:, None]                      # [B, S]
    kbH = np.where(keymask, np.float32(0.0), NEG).astype(np.float32)
    kbH = np.ascontiguousarray(kbH.reshape(B, NT, P).transpose(0, 2, 1))
    qmH = keymask.astype(np.float32)
    qmH = np.ascontiguousarray(qmH.reshape(B, NT, P).transpose(0, 2, 1))
    ident = np.eye(P, dtype=np.float32)

    in_maps = []
    for c in range(NCORES):
        sl = slice(c * BPC, (c + 1) * BPC)
        in_maps.append(
            {
                "query": np.ascontiguousarray(query[sl]),
                "context": np.ascontiguousarray(context[sl]),
                "keybias": np.ascontiguousarray(kbH[sl]),
                "qmask": np.ascontiguousarray(qmH[sl]),
                "identity": ident,
                "identity_r": ident,
                "ones": np.ones((P, 2), dtype=np.float32),
            }
        )

    nc = _get_nc()
    res = run_bass_kernel_spmd(nc, in_maps, list(range(NCORES)))
    _NC_CACHE["last_result"] = res
    out = np.concatenate([res.results[c]["out"] for c in range(NCORES)], axis=0)
    scores = np.concatenate(
        [res.results[c]["scores"] for c in range(NCORES)], axis=0
    )
    return out, scores



# revision 10
# speedup vs baseline: 1.9282x; 1.9282x over previous
"""Trainium2 Bass kernel for BiLinearSigmoidAttention (length-sparse, bf16).

Reference math (per batch b, with L = length[b]):
    qn = l2norm(query), cn = l2norm(context)
    raw[q,k] = qn[q] . cn[k]            (masked: k >= L -> -1e30)
    sig = sigmoid(raw)
    den[q] = max(sum_k sig[q,k], 1)
    scores[q,k] = sig[q,k] / den[q]     (rows q >= L zeroed)
    att[q,:] = sum_k scores[q,k] * context[k,:]
    out = concat([qn, att], -1)
returns (out [B,S,2D], scores [B,S,S])

Key structure (8 NeuronCores, data parallel over B=32 -> 4 slots per core):
  - sigmoid(-1e30) == 0, so only the first T_b = ceil(L_b/128) row/col
    tile-blocks of the [S,S] score matrix are nonzero. Batches are sorted
    by T descending and dealt round-robin to cores; slot j of every core
    runs with the baked tile count ts[j] = max T in that deal group.
    Zero regions are DMA'd from a zeroed SBUF tile.
  - all matmuls and PE transposes run in bf16 (tolerance is 2e-2);
    outputs are written bf16 and upcast to fp32 on the host.
  - mm1 computes sigT [k_part, q_free]; the length mask is a per-partition
    bias and the context l2-norm a per-partition scale fused into the
    sigmoid activation.
  - denominator comes from a vector reduce over the PE-transposed score
    block (no ones-matmuls); w = qmask/max(den,1) is applied during the
    scores/attended evictions.
"""

import numpy as np
import ml_dtypes

import concourse.bacc as bacc
import concourse.mybir as mybir
import concourse.tile as tile
from concourse.bass_utils import run_bass_kernel_spmd

B, S, D = 32, 1024, 512
NCORES = 8
BPC = B // NCORES          # batch slots per core
P = 128                    # partitions
NT = S // P                # 8 s-tiles
ND = D // P                # 4 d-chunks
NEG = np.float32(-1e30)

F32 = mybir.dt.float32
BF16 = mybir.dt.bfloat16
import os as _os
PHASE = int(_os.environ.get("KERNEL_PHASE", "4"))
AF = mybir.ActivationFunctionType
AX = mybir.AxisListType


def build_kernel(ts):
    """ts: per-slot baked tile counts (len BPC, descending, each 1..NT)."""
    nc = bacc.Bacc("TRN2", target_bir_lowering=False, debug=False)

    q_d = nc.dram_tensor("query", [BPC, S, D], F32, kind="ExternalInput")
    c_d = nc.dram_tensor("context", [BPC, S, D], F32, kind="ExternalInput")
    # keybias[b, p, kt] = 0 if kt*P+p < L else -1e30
    kb_d = nc.dram_tensor("keybias", [BPC, P, NT], F32, kind="ExternalInput")
    # qmask[b, p, qb] = 1 if qb*P+p < L else 0
    qm_d = nc.dram_tensor("qmask", [BPC, P, NT], F32, kind="ExternalInput")
    id_d = nc.dram_tensor("identity", [P, P], BF16, kind="ExternalInput")
    out_d = nc.dram_tensor("out", [BPC, S, 2 * D], BF16, kind="ExternalOutput")
    sc_d = nc.dram_tensor("scores", [BPC, S, S], BF16, kind="ExternalOutput")

    with tile.TileContext(nc) as tc:
        _body(tc, ts, q_d, c_d, kb_d, qm_d, id_d, out_d, sc_d)
    nc.compile()
    return nc


def _body(tc, ts, q_d, c_d, kb_d, qm_d, id_d, out_d, sc_d):
    nc = tc.nc
    from contextlib import ExitStack

    ctx = ExitStack()
    with ctx:
        const = ctx.enter_context(tc.tile_pool(name="const", bufs=1))
        qpool = ctx.enter_context(tc.tile_pool(name="q", bufs=2))
        cpool = ctx.enter_context(tc.tile_pool(name="c", bufs=2))
        qbp = ctx.enter_context(tc.tile_pool(name="qb", bufs=2))
        cbp = ctx.enter_context(tc.tile_pool(name="cb", bufs=2))
        tp = ctx.enter_context(tc.tile_pool(name="t", bufs=2))
        sgp = ctx.enter_context(tc.tile_pool(name="sg", bufs=2))
        mpool = ctx.enter_context(tc.tile_pool(name="m", bufs=2))
        spool = ctx.enter_context(tc.tile_pool(name="s", bufs=2))
        opool = ctx.enter_context(tc.tile_pool(name="o", bufs=3))
        ps1 = ctx.enter_context(tc.tile_pool(name="ps1", bufs=2, space="PSUM"))
        pst = ctx.enter_context(tc.tile_pool(name="pst", bufs=2, space="PSUM"))
        ps2 = ctx.enter_context(tc.tile_pool(name="ps2", bufs=2, space="PSUM"))

        idb = const.tile([P, P], BF16, tag="idb")
        nc.sync.dma_start(idb[:], id_d[:])
        zt = const.tile([P, S], BF16, tag="zt")
        nc.gpsimd.memset(zt[:], 0.0)

        for b in range(BPC):
            T = ts[b]
            W = T * P                      # active score width
            NQC = (W + 511) // 512         # 512-col q chunks for mm1

            # ---- zero fills for this slot (no deps; overlap with compute) ----
            for qt in range(T, NT):
                nc.sync.dma_start(sc_d[b, qt * P : (qt + 1) * P, :], zt[:])
                nc.sync.dma_start(
                    out_d[b, qt * P : (qt + 1) * P, D : 2 * D], zt[:, 0:D]
                )

            # ---- loads (sync queue: inputs only, never blocked) ----
            qt_t = qpool.tile([P, NT, D], F32, tag="qt")
            ct_t = cpool.tile([P, NT, D], F32, tag="ct")
            kb = mpool.tile([P, NT], F32, tag="kb")
            qm = mpool.tile([P, NT], F32, tag="qm")
            nc.sync.dma_start(qt_t[:], q_d[b].rearrange("(t p) d -> p t d", p=P))
            nc.sync.dma_start(
                ct_t[:, 0:T], c_d[b, 0:W].rearrange("(t p) d -> p t d", p=P)
            )
            nc.sync.dma_start(kb[:], kb_d[b])
            nc.sync.dma_start(qm[:], qm_d[b])

            # ---- norms: q on vector (fused mul+reduce), c on scalar ----
            ssq = mpool.tile([P, 2 * NT], F32, tag="ssq")
            for t in range(NT):
                scr = spool.tile([P, D], F32, tag="scr")
                nc.vector.tensor_mul(scr[:], qt_t[:, t], qt_t[:, t])
                nc.vector.reduce_sum(ssq[:, t : t + 1], scr[:], axis=AX.X)
            for t in range(T):
                scr2 = spool.tile([P, D], BF16, tag="scr2")
                nc.scalar.activation(
                    scr2[:], ct_t[:, t], AF.Square,
                    accum_out=ssq[:, NT + t : NT + t + 1],
                )
            nrm = mpool.tile([P, 2 * NT], F32, tag="nrm")
            inv = mpool.tile([P, 2 * NT], F32, tag="inv")
            nc.scalar.activation(nrm[:, 0 : NT + T], ssq[:, 0 : NT + T], AF.Sqrt)
            nc.vector.reciprocal(inv[:, 0 : NT + T], nrm[:, 0 : NT + T])

            # ---- qn (bf16) + first half of out; c cast to bf16 ----
            qnb = qbp.tile([P, NT, D], BF16, tag="qnb")
            for t in range(NT):
                nc.vector.tensor_scalar_mul(qnb[:, t], qt_t[:, t], inv[:, t : t + 1])
            nc.sync.dma_start(
                out_d[b, :, 0:D].rearrange("(t p) d -> p t d", p=P), qnb[:]
            )
            cbt = cbp.tile([P, NT, D], BF16, tag="cbt")
            for t in range(T):
                nc.vector.tensor_copy(cbt[:, t], ct_t[:, t])

            if PHASE < 2:
                continue
            # ---- transposes: qT[d, q<W] and cT[d, k<W] (bf16) ----
            qT = tp.tile([P, ND, S], BF16, tag="qT")
            cT = tp.tile([P, ND, S], BF16, tag="cT")
            for t in range(T):
                pq = pst.tile([P, ND, P], BF16, tag="pt")
                pc = pst.tile([P, ND, P], BF16, tag="pt")
                for dch in range(ND):
                    nc.tensor.transpose(
                        pq[:, dch], qnb[:, t, dch * P : (dch + 1) * P], idb[:]
                    )
                    nc.tensor.transpose(
                        pc[:, dch], cbt[:, t, dch * P : (dch + 1) * P], idb[:]
                    )
                nc.scalar.copy(qT[:, :, t * P : (t + 1) * P], pq[:])
                nc.vector.tensor_copy(cT[:, :, t * P : (t + 1) * P], pc[:])

            if PHASE < 3:
                continue
            # ---- mm1: sigT[k, q] = sigmoid(inv_c[k] * (cT.T @ qT) + keybias) ----
            sg = sgp.tile([P, NT, S], BF16, tag="sg")
            for kt in range(T):
                for qc in range(NQC):
                    wq = min(512, W - qc * 512)
                    acc = ps1.tile([P, 512], F32, tag="acc")
                    for dch in range(ND):
                        nc.tensor.matmul(
                            acc[:, 0:wq],
                            cT[:, dch, kt * P : (kt + 1) * P],
                            qT[:, dch, qc * 512 : qc * 512 + wq],
                            start=(dch == 0),
                            stop=(dch == ND - 1),
                        )
                    nc.scalar.activation(
                        sg[:, kt, qc * 512 : qc * 512 + wq], acc[:, 0:wq],
                        AF.Sigmoid, bias=kb[:, kt : kt + 1],
                        scale=inv[:, NT + kt : NT + kt + 1],
                    )

            if PHASE < 4:
                continue
            # ---- per q-block: transpose scores, den, w, attended ----
            for qb in range(T):
                att = ps2.tile([P, 512], F32, tag="att")
                for kt in range(T):
                    nc.tensor.matmul(
                        att[:], sg[:, kt, qb * P : (qb + 1) * P], cbt[:, kt],
                        start=(kt == 0), stop=(kt == T - 1),
                    )

                so = opool.tile([P, W], BF16, tag="so")
                if W < S:
                    # zero tail columns straight from the zero tile (no deps)
                    nc.sync.dma_start(
                        sc_d[b, qb * P : (qb + 1) * P, W:S], zt[:, 0 : S - W]
                    )

                NKG = (T + 3) // 4
                for kg in range(NKG):
                    G = min(4, T - kg * 4)
                    pt = pst.tile([P, ND, P], BF16, tag="pt")
                    for j in range(G):
                        kt = kg * 4 + j
                        nc.tensor.transpose(
                            pt[:, j], sg[:, kt, qb * P : (qb + 1) * P], idb[:]
                        )
                    # evict unscaled sigT^T into so head (alternate engines)
                    if kg % 2 == 0:
                        nc.scalar.copy(
                            so[:, kg * 512 : kg * 512 + G * P], pt[:, 0:G]
                        )
                    else:
                        nc.vector.tensor_copy(
                            so[:, kg * 512 : kg * 512 + G * P], pt[:, 0:G]
                        )

                # den = sum_k sig (bf16 head), w = qmask / max(den, 1)
                den = mpool.tile([P, 1], F32, tag="den")
                w = mpool.tile([P, 1], F32, tag="w")
                nc.vector.reduce_sum(den[:], so[:, 0:W], axis=AX.X)
                nc.vector.tensor_scalar_max(den[:], den[:], 1.0)
                nc.vector.reciprocal(w[:], den[:])
                nc.vector.tensor_mul(w[:], w[:], qm[:, qb : qb + 1])

                # scale scores in place, write out
                nc.vector.tensor_scalar_mul(so[:, 0:W], so[:, 0:W], w[:])
                nc.sync.dma_start(sc_d[b, qb * P : (qb + 1) * P, 0:W], so[:])

                ao = opool.tile([P, D], BF16, tag="ao")
                nc.scalar.mul(ao[:], att[:], w[:])
                nc.sync.dma_start(
                    out_d[b, qb * P : (qb + 1) * P, D : 2 * D], ao[:]
                )


_NC_CACHE = {}


def _get_nc(ts):
    key = ("nc", ts)
    if key not in _NC_CACHE:
        _NC_CACHE[key] = build_kernel(ts)
    return _NC_CACHE[key]


def plan(length):
    """Sort batches by tile count desc, deal round-robin to cores.

    Returns (ts, order): ts[j] = baked tile count for slot j; order[j*NCORES+c]
    = batch index placed in slot j of core c.
    """
    length = np.asarray(length).astype(np.int64)
    T = np.ceil(length / P).astype(np.int64)
    order = np.argsort(-T, kind="stable")
    ts = tuple(int(T[order[j * NCORES]]) for j in range(BPC))
    return ts, order


def prep_inputs(context, query, length):
    context = np.ascontiguousarray(np.asarray(context, dtype=np.float32))
    query = np.ascontiguousarray(np.asarray(query, dtype=np.float32))
    length = np.asarray(length).astype(np.int64)
    ts, order = plan(length)

    iot = np.arange(S)
    keymask = iot[None, :] < length[:, None]                      # [B, S]
    kbH = np.where(keymask, np.float32(0.0), NEG).astype(np.float32)
    kbH = np.ascontiguousarray(kbH.reshape(B, NT, P).transpose(0, 2, 1))
    qmH = keymask.astype(np.float32)
    qmH = np.ascontiguousarray(qmH.reshape(B, NT, P).transpose(0, 2, 1))
    idb = np.eye(P, dtype=ml_dtypes.bfloat16)

    in_maps = []
    for c in range(NCORES):
        bidx = [int(order[j * NCORES + c]) for j in range(BPC)]
        in_maps.append(
            {
                "query": np.ascontiguousarray(query[bidx]),
                "context": np.ascontiguousarray(context[bidx]),
                "keybias": np.ascontiguousarray(kbH[bidx]),
                "qmask": np.ascontiguousarray(qmH[bidx]),
                "identity": idb,
            }
        )
    return ts, order, in_maps


def kernel(context, query, length):
    ts, order, in_maps = prep_inputs(context, query, length)
    nc = _get_nc(ts)
    res = run_bass_kernel_spmd(nc, in_maps, list(range(NCORES)))
    _NC_CACHE["last_result"] = res

    out = np.empty((B, S, 2 * D), np.float32)
    scores = np.empty((B, S, S), np.float32)
    for c in range(NCORES):
        ro = np.asarray(res.results[c]["out"]).astype(np.float32)
        rs = np.asarray(res.results[c]["scores"]).astype(np.float32)
        for j in range(BPC):
            bi = int(order[j * NCORES + c])
            out[bi] = ro[j]
            scores[bi] = rs[j]
    return out, scores
